# revision 4
# baseline (speedup 1.0000x reference)
"""Single-head attention (B=4, S=2048, D=1024) on 8 TRN2 NeuronCores, v3.

Sharding: 8 shards = (batch b, query-half h).  Core c = 2*b + h computes
attention outputs for query rows [h*1024, (h+1)*1024) of batch b.  The host
rotates x per core so the core's query rows are rows [0, 1024); key order is
a permutation, which softmax attention is invariant to, so one SPMD NEFF
serves all 8 cores.

Algebra (v2): scores = x_q (Wq Wk^T) x^T and attn@V = (attn@x) Wv, which
drops per-core matmul work from 19.3 to 15.0 GFLOP with no communication.

v3 moves all data marshalling to the HOST, because per-core HBM input
bandwidth under 8-way SPMD (~110 GB/s) made device-side casts/transposes the
critical path:
  - M = Wq Wk^T is precomputed on host (f32 BLAS, then bf16) -- the device
    M phase and the Wq/Wk tensors disappear entirely.
  - x is shipped twice, pre-cast to bf16: row-major (xb, for the attn@x
    contraction over keys) and pre-transposed (xt, for QK^T's contraction
    over d) -- no device DMA-transposes, no xbar serialization.
  - Wv ships as bf16.
Device inputs: 12 MB instead of 20.6 MB f32 + 8 MB of xbar traffic, with no
load->cast->transpose dependency chains; loads spread over all three DMA
queues (scalar/sync HWDGE + gpsimd SWDGE) in priority order.

Per-core device dataflow (bf16 matmuls, fp32 PSUM, 512-wide moving):
  TT[d',q] = M[d,d'].T-contract xT[d,q]            (PE 128 MM)
  ST[k,q]  = xT[d',k].T-contract TT[d',q]          (PE 256 MM)
  expS     = exp(ST / sqrt(D))                     (ACT)
  UT[e,q]  = x1[k,e].T-contract expS[k,q]          (PE 256 MM)
  den[1,q] = ones[k,1].T-contract expS[k,q]        (PE 32 MM; ones col in x1)
  out[q,e] = (UT[e',q].T-contract Wv[e',e])/den[q] (PE 128 MM + DVE scale)

A warmup accumulation group keeps the PE HAM clock-gate warm while the
loads run, so TT starts at full clock.
"""

import numpy as np

_P = 128


def _build_attention_nc(SQ, S, D, n_cores, warmup_mms=64):
    from contextlib import ExitStack

    import concourse.tile as tile
    import concourse.mybir as mybir
    from concourse import bacc

    f32 = mybir.dt.float32
    bf16 = mybir.dt.bfloat16

    DT = D // _P    # 8  tiles over d / d' / e / e'
    ST = S // _P    # 16 key tiles
    QS = SQ // _P   # 8  query tiles
    HW_ = 512       # moving width (PSUM bank limit for f32 out)
    XW = 1032       # x_bf inner width: 1024 x cols + ones col + pad
    inv_sqrt_d = 1.0 / float(np.sqrt(D))

    nc = bacc.Bacc(
        "TRN2",
        target_bir_lowering=False,
        debug=False,
        enable_asserts=True,
        num_devices=n_cores,
    )
    xb_ap = nc.dram_tensor("xb", [S, D], bf16, kind="ExternalInput").ap()
    xt_ap = nc.dram_tensor("xt", [D, S], bf16, kind="ExternalInput").ap()
    m_ap = nc.dram_tensor("m", [D, D], bf16, kind="ExternalInput").ap()
    wv_ap = nc.dram_tensor("wv", [D, D], bf16, kind="ExternalInput").ap()
    out_ap = nc.dram_tensor("out", [SQ, D], f32, kind="ExternalOutput").ap()

    with ExitStack() as ctx:
        tc = ctx.enter_context(tile.TileContext(nc))

        pers = ctx.enter_context(tc.tile_pool(name="pers", bufs=1))
        x_bf = pers.tile([_P, ST, XW], bf16)     # [k_inner, k_tile, e | ones]
        xT = pers.tile([_P, DT, S], bf16)        # [d_inner, d_tile, s]
        Msb = pers.tile([_P, DT, D], bf16)       # [d_inner, d_tile, d']
        Wv = pers.tile([_P, DT, D], bf16)        # [e'_inner, e'_tile, e]
        TT = pers.tile([_P, DT, SQ], bf16)       # [d'_inner, d'_tile, q]
        warm = pers.tile([_P, HW_], bf16)

        nc.vector.memset(warm, 0.0)
        nc.vector.memset(x_bf[:, :, D : D + 1], 1.0)   # ones column

        psum = ctx.enter_context(tc.tile_pool(name="psum", bufs=4, space="PSUM"))
        psum_dn = ctx.enter_context(tc.tile_pool(name="psum_dn", bufs=1, space="PSUM"))

        # PE warmup: one long accumulation group (no per-MM drain) keeps the
        # HAM clock-gate warm while the inputs load (~259ns per 512-wide MM).
        wps = psum.tile([_P, HW_], f32, tag="mm", name="wps")
        for i in range(warmup_mms):
            nc.tensor.matmul(
                wps, lhsT=warm[:, 0:_P], rhs=warm,
                start=(i == 0), stop=(i == warmup_mms - 1),
            )

        # ---- loads: 3 queues in parallel, priority order ---------------------
        # TT (the first PE phase) needs ALL of xt and M, so those interleave
        # round-robin across all three queues first; xb and Wv follow.
        queues = [nc.scalar, nc.sync, nc.gpsimd]
        qi = 0

        def _ld(out, in_):
            nonlocal qi
            queues[qi % 3].dma_start(out=out, in_=in_)
            qi += 1

        for dt in range(DT):
            _ld(xT[:, dt, :], xt_ap[dt * _P : (dt + 1) * _P, :])
            _ld(Msb[:, dt, :], m_ap[dt * _P : (dt + 1) * _P, :])
        for st in range(ST):
            _ld(x_bf[:, st, 0:D], xb_ap[st * _P : (st + 1) * _P, :])
        for dt in range(DT):
            _ld(Wv[:, dt, :], wv_ap[dt * _P : (dt + 1) * _P, :])

        # ---- TT[d', q] = sum_d M[d, d'] x[q, d] ------------------------------
        for pt in range(DT):
            for h in range(2):
                ps = psum.tile([_P, HW_], f32, tag="mm", name="t_ps")
                for dt in range(DT):
                    nc.tensor.matmul(
                        ps,
                        lhsT=Msb[:, dt, pt * _P : (pt + 1) * _P],
                        rhs=xT[:, dt, h * HW_ : (h + 1) * HW_],
                        start=(dt == 0),
                        stop=(dt == DT - 1),
                    )
                nc.scalar.copy(out=TT[:, pt, h * HW_ : (h + 1) * HW_], in_=ps)

        # ---- scores, exp, U, denominator, output -----------------------------
        with tc.tile_pool(name="att", bufs=1) as att, tc.tile_pool(
            name="outp", bufs=2
        ) as outp:
            expS = att.tile([_P, ST, SQ], bf16)   # [k_inner, k_tile, q]
            U = att.tile([_P, DT, SQ], bf16)      # [e_inner, e_tile, q]
            den_row = att.tile([1, SQ], f32)
            den128 = att.tile([_P, QS], f32)
            recip = att.tile([_P, QS], f32)

            # scores^T[k, q] = sum_d' x[k, d'] T[q, d'];  expS = exp(s / 32)
            for kt in range(ST):
                for h in range(2):
                    ps = psum.tile([_P, HW_], f32, tag="mm", name="s_ps")
                    for pt in range(DT):
                        nc.tensor.matmul(
                            ps,
                            lhsT=xT[:, pt, kt * _P : (kt + 1) * _P],
                            rhs=TT[:, pt, h * HW_ : (h + 1) * HW_],
                            start=(pt == 0),
                            stop=(pt == DT - 1),
                        )
                    nc.scalar.activation(
                        out=expS[:, kt, h * HW_ : (h + 1) * HW_],
                        in_=ps,
                        func=mybir.ActivationFunctionType.Exp,
                        scale=inv_sqrt_d,
                    )

            # U^T[e, q] = sum_k x[k, e] expS[k, q]; den via ones column lhsT
            dn = psum_dn.tile([1, SQ], f32, tag="dn", name="dn_ps")
            for kt in range(ST):
                for h in range(2):
                    nc.tensor.matmul(
                        dn[0:1, h * HW_ : (h + 1) * HW_],
                        lhsT=x_bf[:, kt, D : D + 1],
                        rhs=expS[:, kt, h * HW_ : (h + 1) * HW_],
                        start=(kt == 0),
                        stop=(kt == ST - 1),
                    )
            for et in range(DT):
                for h in range(2):
                    ps = psum.tile([_P, HW_], f32, tag="mm", name="u_ps")
                    for kt in range(ST):
                        nc.tensor.matmul(
                            ps,
                            lhsT=x_bf[:, kt, et * _P : (et + 1) * _P],
                            rhs=expS[:, kt, h * HW_ : (h + 1) * HW_],
                            start=(kt == 0),
                            stop=(kt == ST - 1),
                        )
                    nc.scalar.copy(out=U[:, et, h * HW_ : (h + 1) * HW_], in_=ps)

            nc.scalar.copy(out=den_row, in_=dn)
            # scatter [1, 1024] -> [128, 8]: partition-scatter of contiguous runs
            for qs in range(QS):
                nc.gpsimd.dma_start(
                    out=den128[:, qs : qs + 1],
                    in_=den_row[0:1, qs * _P : (qs + 1) * _P],
                )
            nc.vector.reciprocal(out=recip, in_=den128)

            # out[q, e] = (sum_e' U[q, e'] Wv[e', e]) / den[q]
            for qs in range(QS):
                o_sb = outp.tile([_P, D], f32, tag="o_sb", name="o_sb")
                for h in range(2):
                    ps = psum.tile([_P, HW_], f32, tag="mm", name="o_ps")
                    for et in range(DT):
                        nc.tensor.matmul(
                            ps,
                            lhsT=U[:, et, qs * _P : (qs + 1) * _P],
                            rhs=Wv[:, et, h * HW_ : (h + 1) * HW_],
                            start=(et == 0),
                            stop=(et == DT - 1),
                        )
                    nc.vector.tensor_scalar_mul(
                        out=o_sb[:, h * HW_ : (h + 1) * HW_],
                        in0=ps,
                        scalar1=recip[:, qs : qs + 1],
                    )
                nc.gpsimd.dma_start(
                    out=out_ap[qs * _P : (qs + 1) * _P, :], in_=o_sb
                )

    nc.compile()
    return nc


_NC_CACHE = {}


def _get_nc(SQ, S, D, n_cores):
    key = (SQ, S, D, n_cores)
    if key not in _NC_CACHE:
        _NC_CACHE[key] = _build_attention_nc(SQ, S, D, n_cores)
    return _NC_CACHE[key]


def _shard_inputs(x, w):
    from ml_dtypes import bfloat16

    x = np.ascontiguousarray(np.asarray(x, dtype=np.float32))
    w = np.ascontiguousarray(np.asarray(w, dtype=np.float32))
    B, S, D = x.shape
    n_cores = 8
    halves = n_cores // B
    SQ = S // halves

    m_bf = np.ascontiguousarray((w[0] @ w[1].T).astype(bfloat16))
    wv_bf = np.ascontiguousarray(w[2].astype(bfloat16))

    in_maps = []
    for c in range(n_cores):
        b, h = divmod(c, halves)
        xb = x[b]
        if h:
            xb = np.concatenate([xb[h * SQ :], xb[: h * SQ]], axis=0)
        xb_bf = xb.astype(bfloat16)
        in_maps.append(
            {
                "xb": np.ascontiguousarray(xb_bf),
                "xt": np.ascontiguousarray(xb_bf.T),
                "m": m_bf,
                "wv": wv_bf,
            }
        )
    return in_maps, (B, S, D, n_cores, halves, SQ)


def _run(x, w, **run_kwargs):
    from concourse import bass_utils

    in_maps, (B, S, D, n_cores, halves, SQ) = _shard_inputs(x, w)
    nc = _get_nc(SQ, S, D, n_cores)
    res = bass_utils.run_bass_kernel_spmd(
        nc, in_maps, core_ids=list(range(n_cores)), **run_kwargs
    )
    out = np.empty((B, S, D), dtype=np.float32)
    for c in range(n_cores):
        b, h = divmod(c, halves)
        out[b, h * SQ : (h + 1) * SQ] = res.results[c]["out"]
    return out, res


def kernel(x, kernel):
    """x (4, 2048, 1024) f32, kernel (3, 1024, 1024) f32 -> (4, 2048, 1024) f32."""
    out, _ = _run(x, kernel)
    return out


# revision 7
# speedup vs baseline: 1.1645x; 1.1645x over previous
"""Single-head attention (B=4, S=2048, D=1024) on 8 TRN2 NeuronCores, v3.

Sharding: 8 shards = (batch b, query-half h).  Core c = 2*b + h computes
attention outputs for query rows [h*1024, (h+1)*1024) of batch b.  The host
rotates x per core so the core's query rows are rows [0, 1024); key order is
a permutation, which softmax attention is invariant to, so one SPMD NEFF
serves all 8 cores.

Algebra (v2): scores = x_q (Wq Wk^T) x^T and attn@V = (attn@x) Wv, which
drops per-core matmul work from 19.3 to 15.0 GFLOP with no communication.

v3 moves all data marshalling to the HOST, because per-core HBM input
bandwidth under 8-way SPMD (~110 GB/s) made device-side casts/transposes the
critical path:
  - M = Wq Wk^T is precomputed on host (f32 BLAS, then bf16) -- the device
    M phase and the Wq/Wk tensors disappear entirely.
  - x is shipped twice, pre-cast to bf16: row-major (xb, for the attn@x
    contraction over keys) and pre-transposed (xt, for QK^T's contraction
    over d) -- no device DMA-transposes, no xbar serialization.
  - Wv ships as bf16.
Device inputs: 12 MB instead of 20.6 MB f32 + 8 MB of xbar traffic, with no
load->cast->transpose dependency chains; loads spread over all three DMA
queues (scalar/sync HWDGE + gpsimd SWDGE) in priority order.

Per-core device dataflow (bf16 matmuls, fp32 PSUM, 512-wide moving):
  TT[d',q] = M[d,d'].T-contract xT[d,q]            (PE 128 MM)
  ST[k,q]  = xT[d',k].T-contract TT[d',q]          (PE 256 MM)
  expS     = exp(ST / sqrt(D))                     (ACT)
  UT[e,q]  = x1[k,e].T-contract expS[k,q]          (PE 256 MM)
  den[1,q] = ones[k,1].T-contract expS[k,q]        (PE 32 MM; ones col in x1)
  out[q,e] = (UT[e',q].T-contract Wv[e',e])/den[q] (PE 128 MM + DVE scale)

A warmup accumulation group keeps the PE HAM clock-gate warm while the
loads run, so TT starts at full clock.
"""

import numpy as np

_P = 128


def _build_attention_nc(SQ, S, D, n_cores, warmup_mms=120):
    from contextlib import ExitStack

    import concourse.tile as tile
    import concourse.mybir as mybir
    from concourse import bacc

    f32 = mybir.dt.float32
    bf16 = mybir.dt.bfloat16

    DT = D // _P    # 8  tiles over d / d' / e / e'
    ST = S // _P    # 16 key tiles
    QS = SQ // _P   # 8  query tiles
    HW_ = 512       # moving width (PSUM bank limit for f32 out)
    XW = 1032       # x_bf inner width: 1024 x cols + ones col + pad
    inv_sqrt_d = 1.0 / float(np.sqrt(D))

    nc = bacc.Bacc(
        "TRN2",
        target_bir_lowering=False,
        debug=False,
        enable_asserts=True,
        num_devices=n_cores,
    )
    xb_ap = nc.dram_tensor("xb", [S, D], bf16, kind="ExternalInput").ap()
    xt_ap = nc.dram_tensor("xt", [D, S], bf16, kind="ExternalInput").ap()
    m_ap = nc.dram_tensor("m", [D, D], bf16, kind="ExternalInput").ap()
    wv_ap = nc.dram_tensor("wv", [D, D], bf16, kind="ExternalInput").ap()
    out_ap = nc.dram_tensor("out", [SQ, D], f32, kind="ExternalOutput").ap()

    with ExitStack() as ctx:
        tc = ctx.enter_context(tile.TileContext(nc))

        pers = ctx.enter_context(tc.tile_pool(name="pers", bufs=1))
        x_bf = pers.tile([_P, ST, XW], bf16)     # [k_inner, k_tile, e | ones]
        xT = pers.tile([_P, DT, S], bf16)        # [d_inner, d_tile, s]
        Msb = pers.tile([_P, DT, D], bf16)       # [d_inner, d_tile, d']
        Wv = pers.tile([_P, DT, D], bf16)        # [e'_inner, e'_tile, e]
        TT = pers.tile([_P, DT, SQ], bf16)       # [d'_inner, d'_tile, q]
        warm = pers.tile([_P, HW_], bf16)

        nc.vector.memset(warm, 0.0)
        nc.vector.memset(x_bf[:, :, D : D + 1], 1.0)   # ones column

        psum = ctx.enter_context(tc.tile_pool(name="psum", bufs=4, space="PSUM"))
        psum_dn = ctx.enter_context(tc.tile_pool(name="psum_dn", bufs=1, space="PSUM"))

        # PE warmup: one long accumulation group (no per-MM drain) keeps the
        # HAM clock-gate warm while the inputs load (~259ns per 512-wide MM).
        wps = psum.tile([_P, HW_], f32, tag="mm", name="wps")
        for i in range(warmup_mms):
            nc.tensor.matmul(
                wps, lhsT=warm[:, 0:_P], rhs=warm,
                start=(i == 0), stop=(i == warmup_mms - 1),
            )

        # ---- loads: 3 queues in parallel, priority order ---------------------
        # scalar: M tiles, then own-half xb, then half of Wv
        for dt in range(DT):
            nc.scalar.dma_start(
                out=Msb[:, dt, :], in_=m_ap[dt * _P : (dt + 1) * _P, :]
            )
        for st in range(QS):
            nc.scalar.dma_start(
                out=x_bf[:, st, 0:D], in_=xb_ap[st * _P : (st + 1) * _P, :]
            )
        for dt in range(0, DT // 2):
            nc.scalar.dma_start(
                out=Wv[:, dt, :], in_=wv_ap[dt * _P : (dt + 1) * _P, :]
            )
        # sync: xt tiles (needed first by TT and scores)
        for dt in range(DT):
            nc.sync.dma_start(
                out=xT[:, dt, :], in_=xt_ap[dt * _P : (dt + 1) * _P, :]
            )
        # gpsimd: other-half xb, rest of Wv
        for st in range(QS, ST):
            nc.gpsimd.dma_start(
                out=x_bf[:, st, 0:D], in_=xb_ap[st * _P : (st + 1) * _P, :]
            )
        for dt in range(DT // 2, DT):
            nc.gpsimd.dma_start(
                out=Wv[:, dt, :], in_=wv_ap[dt * _P : (dt + 1) * _P, :]
            )

        # ---- TT[d', q] = sum_d M[d, d'] x[q, d] ------------------------------
        for pt in range(DT):
            for h in range(2):
                ps = psum.tile([_P, HW_], f32, tag="mm", name="t_ps")
                for dt in range(DT):
                    nc.tensor.matmul(
                        ps,
                        lhsT=Msb[:, dt, pt * _P : (pt + 1) * _P],
                        rhs=xT[:, dt, h * HW_ : (h + 1) * HW_],
                        start=(dt == 0),
                        stop=(dt == DT - 1),
                    )
                nc.scalar.copy(out=TT[:, pt, h * HW_ : (h + 1) * HW_], in_=ps)

        # ---- scores, exp, U, denominator, output -----------------------------
        with tc.tile_pool(name="att", bufs=1) as att, tc.tile_pool(
            name="outp", bufs=2
        ) as outp:
            expS = att.tile([_P, ST, SQ], bf16)   # [k_inner, k_tile, q]
            U = att.tile([_P, DT, SQ], bf16)      # [e_inner, e_tile, q]
            den_row = att.tile([1, SQ], f32)
            den128 = att.tile([_P, QS], f32)
            recip = att.tile([_P, QS], f32)

            # scores^T[k, q] = sum_d' x[k, d'] T[q, d'];  expS = exp(s / 32)
            for kt in range(ST):
                for h in range(2):
                    ps = psum.tile([_P, HW_], f32, tag="mm", name="s_ps")
                    for pt in range(DT):
                        nc.tensor.matmul(
                            ps,
                            lhsT=xT[:, pt, kt * _P : (kt + 1) * _P],
                            rhs=TT[:, pt, h * HW_ : (h + 1) * HW_],
                            start=(pt == 0),
                            stop=(pt == DT - 1),
                        )
                    nc.scalar.activation(
                        out=expS[:, kt, h * HW_ : (h + 1) * HW_],
                        in_=ps,
                        func=mybir.ActivationFunctionType.Exp,
                        scale=inv_sqrt_d,
                    )

            # U^T[e, q] = sum_k x[k, e] expS[k, q]; den via ones column lhsT
            dn = psum_dn.tile([1, SQ], f32, tag="dn", name="dn_ps")
            for kt in range(ST):
                for h in range(2):
                    nc.tensor.matmul(
                        dn[0:1, h * HW_ : (h + 1) * HW_],
                        lhsT=x_bf[:, kt, D : D + 1],
                        rhs=expS[:, kt, h * HW_ : (h + 1) * HW_],
                        start=(kt == 0),
                        stop=(kt == ST - 1),
                    )
            for et in range(DT):
                for h in range(2):
                    ps = psum.tile([_P, HW_], f32, tag="mm", name="u_ps")
                    for kt in range(ST):
                        nc.tensor.matmul(
                            ps,
                            lhsT=x_bf[:, kt, et * _P : (et + 1) * _P],
                            rhs=expS[:, kt, h * HW_ : (h + 1) * HW_],
                            start=(kt == 0),
                            stop=(kt == ST - 1),
                        )
                    nc.scalar.copy(out=U[:, et, h * HW_ : (h + 1) * HW_], in_=ps)

            nc.scalar.copy(out=den_row, in_=dn)
            # scatter [1, 1024] -> [128, 8]: partition-scatter of contiguous runs
            for qs in range(QS):
                nc.gpsimd.dma_start(
                    out=den128[:, qs : qs + 1],
                    in_=den_row[0:1, qs * _P : (qs + 1) * _P],
                )
            nc.vector.reciprocal(out=recip, in_=den128)

            # out[q, e] = (sum_e' U[q, e'] Wv[e', e]) / den[q]
            # stores: 256KB halves round-robin over all three queues so the
            # final store drains fast instead of queueing on one SWDGE queue
            st_queues = [nc.sync, nc.scalar, nc.gpsimd]
            for qs in range(QS):
                o_sb = outp.tile([_P, D], f32, tag="o_sb", name="o_sb")
                for h in range(2):
                    ps = psum.tile([_P, HW_], f32, tag="mm", name="o_ps")
                    for et in range(DT):
                        nc.tensor.matmul(
                            ps,
                            lhsT=U[:, et, qs * _P : (qs + 1) * _P],
                            rhs=Wv[:, et, h * HW_ : (h + 1) * HW_],
                            start=(et == 0),
                            stop=(et == DT - 1),
                        )
                    nc.vector.tensor_scalar_mul(
                        out=o_sb[:, h * HW_ : (h + 1) * HW_],
                        in0=ps,
                        scalar1=recip[:, qs : qs + 1],
                    )
                    st_queues[(2 * qs + h) % 3].dma_start(
                        out=out_ap[
                            qs * _P : (qs + 1) * _P, h * HW_ : (h + 1) * HW_
                        ],
                        in_=o_sb[:, h * HW_ : (h + 1) * HW_],
                    )

    nc.compile()
    return nc


_NC_CACHE = {}


def _get_nc(SQ, S, D, n_cores):
    key = (SQ, S, D, n_cores)
    if key not in _NC_CACHE:
        _NC_CACHE[key] = _build_attention_nc(SQ, S, D, n_cores)
    return _NC_CACHE[key]


def _shard_inputs(x, w):
    from ml_dtypes import bfloat16

    x = np.ascontiguousarray(np.asarray(x, dtype=np.float32))
    w = np.ascontiguousarray(np.asarray(w, dtype=np.float32))
    B, S, D = x.shape
    n_cores = 8
    halves = n_cores // B
    SQ = S // halves

    m_bf = np.ascontiguousarray((w[0] @ w[1].T).astype(bfloat16))
    wv_bf = np.ascontiguousarray(w[2].astype(bfloat16))

    in_maps = []
    for c in range(n_cores):
        b, h = divmod(c, halves)
        xb = x[b]
        if h:
            xb = np.concatenate([xb[h * SQ :], xb[: h * SQ]], axis=0)
        xb_bf = xb.astype(bfloat16)
        in_maps.append(
            {
                "xb": np.ascontiguousarray(xb_bf),
                "xt": np.ascontiguousarray(xb_bf.T),
                "m": m_bf,
                "wv": wv_bf,
            }
        )
    return in_maps, (B, S, D, n_cores, halves, SQ)


def _run(x, w, **run_kwargs):
    from concourse import bass_utils

    in_maps, (B, S, D, n_cores, halves, SQ) = _shard_inputs(x, w)
    nc = _get_nc(SQ, S, D, n_cores)
    res = bass_utils.run_bass_kernel_spmd(
        nc, in_maps, core_ids=list(range(n_cores)), **run_kwargs
    )
    out = np.empty((B, S, D), dtype=np.float32)
    for c in range(n_cores):
        b, h = divmod(c, halves)
        out[b, h * SQ : (h + 1) * SQ] = res.results[c]["out"]
    return out, res


def kernel(x, kernel):
    """x (4, 2048, 1024) f32, kernel (3, 1024, 1024) f32 -> (4, 2048, 1024) f32."""
    out, _ = _run(x, kernel)
    return out


# revision 9
# speedup vs baseline: 1.1812x; 1.0144x over previous
"""Single-head attention (B=4, S=2048, D=1024) on 8 TRN2 NeuronCores, v3.

Sharding: 8 shards = (batch b, query-half h).  Core c = 2*b + h computes
attention outputs for query rows [h*1024, (h+1)*1024) of batch b.  The host
rotates x per core so the core's query rows are rows [0, 1024); key order is
a permutation, which softmax attention is invariant to, so one SPMD NEFF
serves all 8 cores.

Algebra (v2): scores = x_q (Wq Wk^T) x^T and attn@V = (attn@x) Wv, which
drops per-core matmul work from 19.3 to 15.0 GFLOP with no communication.

v3 moves all data marshalling to the HOST, because per-core HBM input
bandwidth under 8-way SPMD (~110 GB/s) made device-side casts/transposes the
critical path:
  - M = Wq Wk^T is precomputed on host (f32 BLAS, then bf16) -- the device
    M phase and the Wq/Wk tensors disappear entirely.
  - x is shipped twice, pre-cast to bf16: row-major (xb, for the attn@x
    contraction over keys) and pre-transposed (xt, for QK^T's contraction
    over d) -- no device DMA-transposes, no xbar serialization.
  - Wv ships as bf16.
Device inputs: 12 MB instead of 20.6 MB f32 + 8 MB of xbar traffic, with no
load->cast->transpose dependency chains; loads spread over all three DMA
queues (scalar/sync HWDGE + gpsimd SWDGE) in priority order.

Per-core device dataflow (bf16 matmuls, fp32 PSUM, 512-wide moving):
  TT[d',q] = M[d,d'].T-contract xT[d,q]            (PE 128 MM)
  ST[k,q]  = xT[d',k].T-contract TT[d',q]          (PE 256 MM)
  expS     = exp(ST / sqrt(D))                     (ACT)
  UT[e,q]  = x1[k,e].T-contract expS[k,q]          (PE 256 MM)
  den[1,q] = ones[k,1].T-contract expS[k,q]        (PE 32 MM; ones col in x1)
  out[q,e] = (UT[e',q].T-contract Wv[e',e])/den[q] (PE 128 MM + DVE scale)

A warmup accumulation group keeps the PE HAM clock-gate warm while the
loads run, so TT starts at full clock.
"""

import numpy as np

_P = 128


def _build_attention_nc(SQ, S, D, n_cores, warmup_mms=95):
    from contextlib import ExitStack

    import concourse.tile as tile
    import concourse.mybir as mybir
    from concourse import bacc

    f32 = mybir.dt.float32
    bf16 = mybir.dt.bfloat16

    DT = D // _P    # 8  tiles over d / d' / e / e'
    ST = S // _P    # 16 key tiles
    QS = SQ // _P   # 8  query tiles
    HW_ = 512       # moving width (PSUM bank limit for f32 out)
    XW = 1032       # x_bf inner width: 1024 x cols + ones col + pad
    inv_sqrt_d = 1.0 / float(np.sqrt(D))

    nc = bacc.Bacc(
        "TRN2",
        target_bir_lowering=False,
        debug=False,
        enable_asserts=True,
        num_devices=n_cores,
    )
    xb_ap = nc.dram_tensor("xb", [S, D], bf16, kind="ExternalInput").ap()
    xt_ap = nc.dram_tensor("xt", [D, S], bf16, kind="ExternalInput").ap()
    m_ap = nc.dram_tensor("m", [D, D], bf16, kind="ExternalInput").ap()
    wv_ap = nc.dram_tensor("wv", [D, D], bf16, kind="ExternalInput").ap()
    out_ap = nc.dram_tensor("out", [SQ, D], f32, kind="ExternalOutput").ap()

    with ExitStack() as ctx:
        tc = ctx.enter_context(tile.TileContext(nc))

        pers = ctx.enter_context(tc.tile_pool(name="pers", bufs=1))
        x_bf = pers.tile([_P, ST, XW], bf16)     # [k_inner, k_tile, e | ones]
        xT = pers.tile([_P, DT, S], bf16)        # [d_inner, d_tile, s]
        Msb = pers.tile([_P, DT, D], bf16)       # [d_inner, d_tile, d']
        Wv = pers.tile([_P, DT, D], bf16)        # [e'_inner, e'_tile, e]
        TT = pers.tile([_P, DT, SQ], bf16)       # [d'_inner, d'_tile, q]
        warm = pers.tile([_P, HW_], bf16)

        nc.vector.memset(warm, 0.0)
        nc.vector.memset(x_bf[:, :, D : D + 1], 1.0)   # ones column

        psum = ctx.enter_context(tc.tile_pool(name="psum", bufs=4, space="PSUM"))
        psum_dn = ctx.enter_context(tc.tile_pool(name="psum_dn", bufs=1, space="PSUM"))

        # PE warmup: one long accumulation group (no per-MM drain) keeps the
        # HAM clock-gate warm while the inputs load (~259ns per 512-wide MM).
        wps = psum.tile([_P, HW_], f32, tag="mm", name="wps")
        for i in range(warmup_mms):
            nc.tensor.matmul(
                wps, lhsT=warm[:, 0:_P], rhs=warm,
                start=(i == 0), stop=(i == warmup_mms - 1),
            )

        # ---- loads: 3 queues in parallel, priority order ---------------------
        # TT (first PE phase) needs all of xt and M: xt rides sync, except two
        # tiles pulled to the FRONT of the other queues so the xt tail doesn't
        # gate TT; M leads scalar; xb and Wv fill in behind.
        nc.scalar.dma_start(out=xT[:, 6, :], in_=xt_ap[6 * _P : 7 * _P, :])
        nc.gpsimd.dma_start(out=xT[:, 7, :], in_=xt_ap[7 * _P : 8 * _P, :])
        for dt in range(DT):
            nc.scalar.dma_start(
                out=Msb[:, dt, :], in_=m_ap[dt * _P : (dt + 1) * _P, :]
            )
        for st in range(QS):
            nc.scalar.dma_start(
                out=x_bf[:, st, 0:D], in_=xb_ap[st * _P : (st + 1) * _P, :]
            )
        for dt in range(0, DT // 2):
            nc.scalar.dma_start(
                out=Wv[:, dt, :], in_=wv_ap[dt * _P : (dt + 1) * _P, :]
            )
        for dt in range(6):
            nc.sync.dma_start(
                out=xT[:, dt, :], in_=xt_ap[dt * _P : (dt + 1) * _P, :]
            )
        for st in range(QS, ST):
            nc.gpsimd.dma_start(
                out=x_bf[:, st, 0:D], in_=xb_ap[st * _P : (st + 1) * _P, :]
            )
        for dt in range(DT // 2, DT):
            nc.gpsimd.dma_start(
                out=Wv[:, dt, :], in_=wv_ap[dt * _P : (dt + 1) * _P, :]
            )

        # ---- TT[d', q] = sum_d M[d, d'] x[q, d] ------------------------------
        for pt in range(DT):
            for h in range(2):
                ps = psum.tile([_P, HW_], f32, tag="mm", name="t_ps")
                for dt in range(DT):
                    nc.tensor.matmul(
                        ps,
                        lhsT=Msb[:, dt, pt * _P : (pt + 1) * _P],
                        rhs=xT[:, dt, h * HW_ : (h + 1) * HW_],
                        start=(dt == 0),
                        stop=(dt == DT - 1),
                    )
                nc.scalar.copy(out=TT[:, pt, h * HW_ : (h + 1) * HW_], in_=ps)

        # ---- scores, exp, U, denominator, output -----------------------------
        with tc.tile_pool(name="att", bufs=1) as att, tc.tile_pool(
            name="outp", bufs=2
        ) as outp:
            expS = att.tile([_P, ST, SQ], bf16)   # [k_inner, k_tile, q]
            U = att.tile([_P, DT, SQ], bf16)      # [e_inner, e_tile, q]
            den_row = att.tile([1, SQ], f32)
            den128 = att.tile([_P, QS], f32)
            recip = att.tile([_P, QS], f32)

            # scores^T[k, q] = sum_d' x[k, d'] T[q, d'];  expS = exp(s / 32)
            for kt in range(ST):
                for h in range(2):
                    ps = psum.tile([_P, HW_], f32, tag="mm", name="s_ps")
                    for pt in range(DT):
                        nc.tensor.matmul(
                            ps,
                            lhsT=xT[:, pt, kt * _P : (kt + 1) * _P],
                            rhs=TT[:, pt, h * HW_ : (h + 1) * HW_],
                            start=(pt == 0),
                            stop=(pt == DT - 1),
                        )
                    nc.scalar.activation(
                        out=expS[:, kt, h * HW_ : (h + 1) * HW_],
                        in_=ps,
                        func=mybir.ActivationFunctionType.Exp,
                        scale=inv_sqrt_d,
                    )

            # U^T[e, q] = sum_k x[k, e] expS[k, q]; den via ones column lhsT
            dn = psum_dn.tile([1, SQ], f32, tag="dn", name="dn_ps")
            for kt in range(ST):
                for h in range(2):
                    nc.tensor.matmul(
                        dn[0:1, h * HW_ : (h + 1) * HW_],
                        lhsT=x_bf[:, kt, D : D + 1],
                        rhs=expS[:, kt, h * HW_ : (h + 1) * HW_],
                        start=(kt == 0),
                        stop=(kt == ST - 1),
                    )
            for et in range(DT):
                for h in range(2):
                    ps = psum.tile([_P, HW_], f32, tag="mm", name="u_ps")
                    for kt in range(ST):
                        nc.tensor.matmul(
                            ps,
                            lhsT=x_bf[:, kt, et * _P : (et + 1) * _P],
                            rhs=expS[:, kt, h * HW_ : (h + 1) * HW_],
                            start=(kt == 0),
                            stop=(kt == ST - 1),
                        )
                    nc.scalar.copy(out=U[:, et, h * HW_ : (h + 1) * HW_], in_=ps)

            nc.scalar.copy(out=den_row, in_=dn)
            # scatter [1, 1024] -> [128, 8]: partition-scatter of contiguous runs
            for qs in range(QS):
                nc.gpsimd.dma_start(
                    out=den128[:, qs : qs + 1],
                    in_=den_row[0:1, qs * _P : (qs + 1) * _P],
                )
            nc.vector.reciprocal(out=recip, in_=den128)

            # out[q, e] = (sum_e' U[q, e'] Wv[e', e]) / den[q]
            # stores: 256KB halves round-robin over all three queues so the
            # final store drains fast instead of queueing on one SWDGE queue
            st_queues = [nc.sync, nc.scalar, nc.gpsimd]
            for qs in range(QS):
                o_sb = outp.tile([_P, D], f32, tag="o_sb", name="o_sb")
                for h in range(2):
                    ps = psum.tile([_P, HW_], f32, tag="mm", name="o_ps")
                    for et in range(DT):
                        nc.tensor.matmul(
                            ps,
                            lhsT=U[:, et, qs * _P : (qs + 1) * _P],
                            rhs=Wv[:, et, h * HW_ : (h + 1) * HW_],
                            start=(et == 0),
                            stop=(et == DT - 1),
                        )
                    nc.vector.tensor_scalar_mul(
                        out=o_sb[:, h * HW_ : (h + 1) * HW_],
                        in0=ps,
                        scalar1=recip[:, qs : qs + 1],
                    )
                    st_queues[(2 * qs + h) % 3].dma_start(
                        out=out_ap[
                            qs * _P : (qs + 1) * _P, h * HW_ : (h + 1) * HW_
                        ],
                        in_=o_sb[:, h * HW_ : (h + 1) * HW_],
                    )

    nc.compile()
    return nc


_NC_CACHE = {}


def _get_nc(SQ, S, D, n_cores):
    key = (SQ, S, D, n_cores)
    if key not in _NC_CACHE:
        _NC_CACHE[key] = _build_attention_nc(SQ, S, D, n_cores)
    return _NC_CACHE[key]


def _shard_inputs(x, w):
    from ml_dtypes import bfloat16

    x = np.ascontiguousarray(np.asarray(x, dtype=np.float32))
    w = np.ascontiguousarray(np.asarray(w, dtype=np.float32))
    B, S, D = x.shape
    n_cores = 8
    halves = n_cores // B
    SQ = S // halves

    m_bf = np.ascontiguousarray((w[0] @ w[1].T).astype(bfloat16))
    wv_bf = np.ascontiguousarray(w[2].astype(bfloat16))

    in_maps = []
    for c in range(n_cores):
        b, h = divmod(c, halves)
        xb = x[b]
        if h:
            xb = np.concatenate([xb[h * SQ :], xb[: h * SQ]], axis=0)
        xb_bf = xb.astype(bfloat16)
        in_maps.append(
            {
                "xb": np.ascontiguousarray(xb_bf),
                "xt": np.ascontiguousarray(xb_bf.T),
                "m": m_bf,
                "wv": wv_bf,
            }
        )
    return in_maps, (B, S, D, n_cores, halves, SQ)


def _run(x, w, **run_kwargs):
    from concourse import bass_utils

    in_maps, (B, S, D, n_cores, halves, SQ) = _shard_inputs(x, w)
    nc = _get_nc(SQ, S, D, n_cores)
    res = bass_utils.run_bass_kernel_spmd(
        nc, in_maps, core_ids=list(range(n_cores)), **run_kwargs
    )
    out = np.empty((B, S, D), dtype=np.float32)
    for c in range(n_cores):
        b, h = divmod(c, halves)
        out[b, h * SQ : (h + 1) * SQ] = res.results[c]["out"]
    return out, res


def kernel(x, kernel):
    """x (4, 2048, 1024) f32, kernel (3, 1024, 1024) f32 -> (4, 2048, 1024) f32."""
    out, _ = _run(x, kernel)
    return out


# revision 11
# speedup vs baseline: 1.1969x; 1.0132x over previous
"""Single-head attention (B=4, S=2048, D=1024) on 8 TRN2 NeuronCores, v3.

Sharding: 8 shards = (batch b, query-half h).  Core c = 2*b + h computes
attention outputs for query rows [h*1024, (h+1)*1024) of batch b.  The host
rotates x per core so the core's query rows are rows [0, 1024); key order is
a permutation, which softmax attention is invariant to, so one SPMD NEFF
serves all 8 cores.

Algebra (v2): scores = x_q (Wq Wk^T) x^T and attn@V = (attn@x) Wv, which
drops per-core matmul work from 19.3 to 15.0 GFLOP with no communication.

v3 moves all data marshalling to the HOST, because per-core HBM input
bandwidth under 8-way SPMD (~110 GB/s) made device-side casts/transposes the
critical path:
  - M = Wq Wk^T is precomputed on host (f32 BLAS, then bf16) -- the device
    M phase and the Wq/Wk tensors disappear entirely.
  - x is shipped twice, pre-cast to bf16: row-major (xb, for the attn@x
    contraction over keys) and pre-transposed (xt, for QK^T's contraction
    over d) -- no device DMA-transposes, no xbar serialization.
  - Wv ships as bf16.
Device inputs: 12 MB instead of 20.6 MB f32 + 8 MB of xbar traffic, with no
load->cast->transpose dependency chains; loads spread over all three DMA
queues (scalar/sync HWDGE + gpsimd SWDGE) in priority order.

Per-core device dataflow (bf16 matmuls, fp32 PSUM, 512-wide moving):
  TT[d',q] = M[d,d'].T-contract xT[d,q]            (PE 128 MM)
  ST[k,q]  = xT[d',k].T-contract TT[d',q]          (PE 256 MM)
  expS     = exp(ST / sqrt(D))                     (ACT)
  UT[e,q]  = x1[k,e].T-contract expS[k,q]          (PE 256 MM)
  den[1,q] = ones[k,1].T-contract expS[k,q]        (PE 32 MM; ones col in x1)
  out[q,e] = (UT[e',q].T-contract Wv[e',e])/den[q] (PE 128 MM + DVE scale)

A warmup accumulation group keeps the PE HAM clock-gate warm while the
loads run, so TT starts at full clock.
"""

import numpy as np

_P = 128


def _build_attention_nc(SQ, S, D, n_cores, warmup_mms=120):
    from contextlib import ExitStack

    import concourse.tile as tile
    import concourse.mybir as mybir
    from concourse import bacc

    f32 = mybir.dt.float32
    bf16 = mybir.dt.bfloat16

    DT = D // _P    # 8  tiles over d / d' / e / e'
    ST = S // _P    # 16 key tiles
    QS = SQ // _P   # 8  query tiles
    HW_ = 512       # moving width (PSUM bank limit for f32 out)
    XW = 1032       # x_bf inner width: 1024 x cols + ones col + pad
    inv_sqrt_d = 1.0 / float(np.sqrt(D))

    nc = bacc.Bacc(
        "TRN2",
        target_bir_lowering=False,
        debug=False,
        enable_asserts=True,
        num_devices=n_cores,
    )
    xb_ap = nc.dram_tensor("xb", [S, D], bf16, kind="ExternalInput").ap()
    xt_ap = nc.dram_tensor("xt", [D, S], bf16, kind="ExternalInput").ap()
    m_ap = nc.dram_tensor("m", [D, D], bf16, kind="ExternalInput").ap()
    wv_ap = nc.dram_tensor("wv", [D, D], bf16, kind="ExternalInput").ap()
    out_ap = nc.dram_tensor("out", [SQ, D], f32, kind="ExternalOutput").ap()

    with ExitStack() as ctx:
        tc = ctx.enter_context(tile.TileContext(nc))

        pers = ctx.enter_context(tc.tile_pool(name="pers", bufs=1))
        x_bf = pers.tile([_P, ST, XW], bf16)     # [k_inner, k_tile, e | ones]
        xT = pers.tile([_P, DT, S], bf16)        # [d_inner, d_tile, s]
        Msb = pers.tile([_P, DT, D], bf16)       # [d_inner, d_tile, d']
        Wv = pers.tile([_P, DT, D], bf16)        # [e'_inner, e'_tile, e]
        TT = pers.tile([_P, DT, SQ], bf16)       # [d'_inner, d'_tile, q]
        warm = pers.tile([_P, HW_], bf16)

        nc.vector.memset(warm, 0.0)
        nc.vector.memset(x_bf[:, :, D : D + 1], 1.0)   # ones column

        psum = ctx.enter_context(tc.tile_pool(name="psum", bufs=4, space="PSUM"))
        psum_dn = ctx.enter_context(tc.tile_pool(name="psum_dn", bufs=1, space="PSUM"))

        # PE warmup: one long accumulation group (no per-MM drain) keeps the
        # HAM clock-gate warm while the inputs load (~259ns per 512-wide MM).
        wps = psum.tile([_P, HW_], f32, tag="mm", name="wps")
        for i in range(warmup_mms):
            nc.tensor.matmul(
                wps, lhsT=warm[:, 0:_P], rhs=warm,
                start=(i == 0), stop=(i == warmup_mms - 1),
            )

        # ---- loads: 3 queues in parallel, priority order ---------------------
        # TT (first PE phase) needs all of xt and M (6MB): balance that
        # critical prefix at ~2MB per queue, then xb, then Wv.
        for dt in range(DT):
            nc.scalar.dma_start(
                out=Msb[:, dt, :], in_=m_ap[dt * _P : (dt + 1) * _P, :]
            )
        for dt in range(4):
            nc.sync.dma_start(
                out=xT[:, dt, :], in_=xt_ap[dt * _P : (dt + 1) * _P, :]
            )
        for dt in range(4, DT):
            nc.gpsimd.dma_start(
                out=xT[:, dt, :], in_=xt_ap[dt * _P : (dt + 1) * _P, :]
            )
        for st in range(QS):
            nc.scalar.dma_start(
                out=x_bf[:, st, 0:D], in_=xb_ap[st * _P : (st + 1) * _P, :]
            )
        for st in range(QS, QS + 4):
            nc.sync.dma_start(
                out=x_bf[:, st, 0:D], in_=xb_ap[st * _P : (st + 1) * _P, :]
            )
        for st in range(QS + 4, ST):
            nc.gpsimd.dma_start(
                out=x_bf[:, st, 0:D], in_=xb_ap[st * _P : (st + 1) * _P, :]
            )
        for dt in range(DT):
            (nc.sync if dt % 2 == 0 else nc.gpsimd).dma_start(
                out=Wv[:, dt, :], in_=wv_ap[dt * _P : (dt + 1) * _P, :]
            )

        # ---- TT[d', q] = sum_d M[d, d'] x[q, d] ------------------------------
        for pt in range(DT):
            for h in range(2):
                ps = psum.tile([_P, HW_], f32, tag="mm", name="t_ps")
                for dt in range(DT):
                    nc.tensor.matmul(
                        ps,
                        lhsT=Msb[:, dt, pt * _P : (pt + 1) * _P],
                        rhs=xT[:, dt, h * HW_ : (h + 1) * HW_],
                        start=(dt == 0),
                        stop=(dt == DT - 1),
                    )
                nc.scalar.copy(out=TT[:, pt, h * HW_ : (h + 1) * HW_], in_=ps)

        # ---- scores, exp, U, denominator, output -----------------------------
        with tc.tile_pool(name="att", bufs=1) as att, tc.tile_pool(
            name="outp", bufs=2
        ) as outp:
            expS = att.tile([_P, ST, SQ], bf16)   # [k_inner, k_tile, q]
            U = att.tile([_P, DT, SQ], bf16)      # [e_inner, e_tile, q]
            den_row = att.tile([1, SQ], f32)
            den128 = att.tile([_P, QS], f32)
            recip = att.tile([_P, QS], f32)

            # scores^T[k, q] = sum_d' x[k, d'] T[q, d'];  expS = exp(s / 32)
            for kt in range(ST):
                for h in range(2):
                    ps = psum.tile([_P, HW_], f32, tag="mm", name="s_ps")
                    for pt in range(DT):
                        nc.tensor.matmul(
                            ps,
                            lhsT=xT[:, pt, kt * _P : (kt + 1) * _P],
                            rhs=TT[:, pt, h * HW_ : (h + 1) * HW_],
                            start=(pt == 0),
                            stop=(pt == DT - 1),
                        )
                    nc.scalar.activation(
                        out=expS[:, kt, h * HW_ : (h + 1) * HW_],
                        in_=ps,
                        func=mybir.ActivationFunctionType.Exp,
                        scale=inv_sqrt_d,
                    )

            # U^T[e, q] = sum_k x[k, e] expS[k, q]; den via ones column lhsT
            dn = psum_dn.tile([1, SQ], f32, tag="dn", name="dn_ps")
            for kt in range(ST):
                for h in range(2):
                    nc.tensor.matmul(
                        dn[0:1, h * HW_ : (h + 1) * HW_],
                        lhsT=x_bf[:, kt, D : D + 1],
                        rhs=expS[:, kt, h * HW_ : (h + 1) * HW_],
                        start=(kt == 0),
                        stop=(kt == ST - 1),
                    )
            for et in range(DT):
                for h in range(2):
                    ps = psum.tile([_P, HW_], f32, tag="mm", name="u_ps")
                    for kt in range(ST):
                        nc.tensor.matmul(
                            ps,
                            lhsT=x_bf[:, kt, et * _P : (et + 1) * _P],
                            rhs=expS[:, kt, h * HW_ : (h + 1) * HW_],
                            start=(kt == 0),
                            stop=(kt == ST - 1),
                        )
                    nc.scalar.copy(out=U[:, et, h * HW_ : (h + 1) * HW_], in_=ps)

            nc.scalar.copy(out=den_row, in_=dn)
            # scatter [1, 1024] -> [128, 8]: partition-scatter of contiguous runs
            for qs in range(QS):
                nc.gpsimd.dma_start(
                    out=den128[:, qs : qs + 1],
                    in_=den_row[0:1, qs * _P : (qs + 1) * _P],
                )
            nc.vector.reciprocal(out=recip, in_=den128)

            # out[q, e] = (sum_e' U[q, e'] Wv[e', e]) / den[q]
            # stores: 256KB halves round-robin over all three queues so the
            # final store drains fast instead of queueing on one SWDGE queue
            st_queues = [nc.sync, nc.scalar, nc.gpsimd]
            for qs in range(QS):
                o_sb = outp.tile([_P, D], f32, tag="o_sb", name="o_sb")
                for h in range(2):
                    ps = psum.tile([_P, HW_], f32, tag="mm", name="o_ps")
                    for et in range(DT):
                        nc.tensor.matmul(
                            ps,
                            lhsT=U[:, et, qs * _P : (qs + 1) * _P],
                            rhs=Wv[:, et, h * HW_ : (h + 1) * HW_],
                            start=(et == 0),
                            stop=(et == DT - 1),
                        )
                    nc.vector.tensor_scalar_mul(
                        out=o_sb[:, h * HW_ : (h + 1) * HW_],
                        in0=ps,
                        scalar1=recip[:, qs : qs + 1],
                    )
                    st_queues[(2 * qs + h) % 3].dma_start(
                        out=out_ap[
                            qs * _P : (qs + 1) * _P, h * HW_ : (h + 1) * HW_
                        ],
                        in_=o_sb[:, h * HW_ : (h + 1) * HW_],
                    )

    nc.compile()
    return nc


_NC_CACHE = {}


def _get_nc(SQ, S, D, n_cores):
    key = (SQ, S, D, n_cores)
    if key not in _NC_CACHE:
        _NC_CACHE[key] = _build_attention_nc(SQ, S, D, n_cores)
    return _NC_CACHE[key]


def _shard_inputs(x, w):
    from ml_dtypes import bfloat16

    x = np.ascontiguousarray(np.asarray(x, dtype=np.float32))
    w = np.ascontiguousarray(np.asarray(w, dtype=np.float32))
    B, S, D = x.shape
    n_cores = 8
    halves = n_cores // B
    SQ = S // halves

    m_bf = np.ascontiguousarray((w[0] @ w[1].T).astype(bfloat16))
    wv_bf = np.ascontiguousarray(w[2].astype(bfloat16))

    in_maps = []
    for c in range(n_cores):
        b, h = divmod(c, halves)
        xb = x[b]
        if h:
            xb = np.concatenate([xb[h * SQ :], xb[: h * SQ]], axis=0)
        xb_bf = xb.astype(bfloat16)
        in_maps.append(
            {
                "xb": np.ascontiguousarray(xb_bf),
                "xt": np.ascontiguousarray(xb_bf.T),
                "m": m_bf,
                "wv": wv_bf,
            }
        )
    return in_maps, (B, S, D, n_cores, halves, SQ)


def _run(x, w, **run_kwargs):
    from concourse import bass_utils

    in_maps, (B, S, D, n_cores, halves, SQ) = _shard_inputs(x, w)
    nc = _get_nc(SQ, S, D, n_cores)
    res = bass_utils.run_bass_kernel_spmd(
        nc, in_maps, core_ids=list(range(n_cores)), **run_kwargs
    )
    out = np.empty((B, S, D), dtype=np.float32)
    for c in range(n_cores):
        b, h = divmod(c, halves)
        out[b, h * SQ : (h + 1) * SQ] = res.results[c]["out"]
    return out, res


def kernel(x, kernel):
    """x (4, 2048, 1024) f32, kernel (3, 1024, 1024) f32 -> (4, 2048, 1024) f32."""
    out, _ = _run(x, kernel)
    return out
